# revision 38
# baseline (speedup 1.0000x reference)
"""Multi-head attention Bass/Tile kernel for Trainium2.

Problem: nn_MultiHeadAttention  (B=8, S=1024, D=768, H=12, HD=64)
  q = x_h @ Wq^T + bq ; k,v likewise (per head)
  scores = q @ k^T        (NO pre-softmax scaling)
  attn = softmax(scores, -1) / sqrt(64)
  out = attn @ v, heads concatenated -> [B, S, D]

Sharding: data parallel over batch, one sample per NeuronCore (8 cores).
HW exec time ~152 us/core (from ~300 us naive-layout first version).

Key design points:
- All big matmuls run as float32r: fp32 storage with ~12-bit mantissa in
  the PE array, 1 cycle/row when the moving dim >= 256 (4x faster than
  plain fp32). End-to-end absmax-relative error ~3.2e-3.
- fp32r ISA restrictions: M (PE columns) must span all 4 column groups
  (>96), even moving-dim counts, 8-byte-aligned single-bank dst.
- The HAM activity monitor throttles the PE clock 2.4->1.2 GHz when the
  array looks half-idle: K=64 contractions read as idle, so everything is
  zero-padded to K=128 (host-side zero rows).
- Layouts are all transposed (computed via PE) so softmax reduces along
  the free axis implicitly: scoresT[t, s] per t-chunk; exp with NO
  max-subtraction (scores ~ N(0,64), exp range safe in fp32); row-sums
  come free as an extra 8.0-scaled ones column in the V weights (also
  folds the post-softmax /sqrt(64)); biases fold in via a ones row in
  x^T (K=65 of 128).
- scoresT needs both Q^T and K^T at partition base 0 with K=128: wqk
  projects [Q^T; K^T] stacked (lhsT chunks), wzq projects [0; Q^T] (the
  moving operand; zero rows annihilate the Q-side of the lhsT chunk).
- attn@V runs in bf16 (exp output + V'): same 1 cycle/row but 1-pass
  LDWEIGHTS; the shared rounding of attn weights in numerator and
  denominator (ones-column row-sums) makes the accuracy cost tiny.
- Software pipeline: per head, scores(h) matmuls (paced by exp freeing
  the 2 PSUM score slots) interleave with attnv(h-1) matmuls, with
  proj(h+1) and output-transpose(h-2) work sprinkled one step per round;
  output columns flush to HBM in chunks as heads complete.
"""

import os
import sys

for _p in (
    "/opt/trn_rl_repo",
    "/root/.axon_site",
    "/root/.axon_site/_ro/trn_rl_repo",
    "/root/.axon_site/_ro/pypackages",
):
    if os.path.isdir(_p) and _p not in sys.path:
        sys.path.append(_p)

import numpy as np

import concourse.bacc as bacc
import concourse.bass as bass
import concourse.tile as tile
from concourse import mybir

B, S, D, H, HD = 8, 1024, 768, 12, 64
K1 = HD + 1  # 65: contraction dim with ones row for bias folding
VW = 66  # V' chunk width (64 e + rowsum col + even pad)
NT = S // 128  # 8 t-chunks / s-chunks
F32 = mybir.dt.float32
F32R = mybir.dt.float32r
BF16 = mybir.dt.bfloat16


def build_nc():
    nc = bacc.Bacc(
        "TRN2",
        target_bir_lowering=False,
        debug=False,
        num_devices=1,
    )

    xt_d = nc.dram_tensor("xt", [H, 128, S], F32R, kind="ExternalInput").ap()
    wqk_d = nc.dram_tensor("wqk", [H, 128, 128], F32R, kind="ExternalInput").ap()
    wzq_d = nc.dram_tensor("wzq", [H, 128, 128], F32R, kind="ExternalInput").ap()
    wv_d = nc.dram_tensor("wv", [H, 128, VW], F32R, kind="ExternalInput").ap()
    ident_d = nc.dram_tensor("ident", [128, 128], F32R, kind="ExternalInput").ap()
    y_d = nc.dram_tensor("y", [S, D], F32, kind="ExternalOutput").ap()

    from contextlib import ExitStack

    with tile.TileContext(nc) as tc:
        with ExitStack() as ctx:
            _emit(ctx, tc, xt_d, wqk_d, wzq_d, wv_d, ident_d, y_d)

    nc.compile()
    return nc


def _emit(ctx, tc, xt_d, wqk_d, wzq_d, wv_d, ident_d, y_d):
    nc = tc.nc
    Exp = mybir.ActivationFunctionType.Exp

    consts = ctx.enter_context(tc.tile_pool(name="consts", bufs=1))
    qkt_pool = ctx.enter_context(tc.tile_pool(name="qkt", bufs=2))
    vp_pool = ctx.enter_context(tc.tile_pool(name="vp", bufs=2))
    attn_pool = ctx.enter_context(tc.tile_pool(name="attn", bufs=16))
    otsb_pool = ctx.enter_context(tc.tile_pool(name="otsb", bufs=2))
    recip_pool = ctx.enter_context(tc.tile_pool(name="recip", bufs=2))
    ps_sc = ctx.enter_context(tc.tile_pool(name="ps_sc", bufs=2, space="PSUM"))
    ps_ot = ctx.enter_context(tc.tile_pool(name="ps_ot", bufs=1, space="PSUM"))
    ps_misc = ctx.enter_context(tc.tile_pool(name="ps_misc", bufs=2, space="PSUM"))

    # ---- constant loads -------------------------------------------------
    wqk_sb = consts.tile([128, H, 128], F32R, name="wqk_sb")
    wzq_sb = consts.tile([128, H, 128], F32R, name="wzq_sb")
    wv_sb = consts.tile([128, H, VW], F32R, name="wv_sb")
    for sb, d in ((wqk_sb, wqk_d), (wzq_sb, wzq_d), (wv_sb, wv_d)):
        dt = d.rearrange("h p j -> p h j")
        nc.gpsimd.dma_start(out=sb[:, 0:1, :], in_=dt[:, 0:1, :])
        nc.gpsimd.dma_start(out=sb[:, 1:H, :], in_=dt[:, 1:H, :])
    # xt loads: heads 0-3 chunked on the SP sequencer (prologue critical
    # path); later heads as single DMAs issued from the idle GpSimd
    # sequencer so neither ACT nor DVE queues stall behind DMA issue.
    xt_sb = []
    for h in range(H):
        t = consts.tile([128, S], F32R, name=f"xt{h}")
        if h == 0:
            # head 0 gates the prologue: 8 chunks split across two issue
            # queues so transfers run on 8 DMA engines in parallel
            for c in range(8):
                eng = nc.sync if c < 4 else nc.scalar
                eng.dma_start(
                    out=t[:, 128 * c : 128 * c + 128],
                    in_=xt_d[h][:, 128 * c : 128 * c + 128],
                )
        elif h < 4:
            for c in range(4):
                nc.sync.dma_start(
                    out=t[:, 256 * c : 256 * c + 256],
                    in_=xt_d[h][:, 256 * c : 256 * c + 256],
                )
        else:
            nc.gpsimd.dma_start(out=t, in_=xt_d[h])
        xt_sb.append(t)

    ident = consts.tile([128, 128], F32R, name="ident")
    nc.gpsimd.dma_start(out=ident, in_=ident_d)

    ysb = consts.tile([128, NT, D], F32, name="ysb")

    # ---- software pipeline over heads ----------------------------------
    at_tiles = {}  # h -> list of 8 attnT sbuf tiles
    vp_sb = {}  # h -> V' sbuf tile [128, 8*66 + pad]
    ot_sb = {}  # h -> OT' sbuf tile [65, 1024]

    def proj_steps(h):
        """Yield small chunks of head-h projection work (Q/K/ZQ/V'), to be
        sprinkled between the exp-paced interleaved rounds."""
        # QK^T = wqk[h].T @ xt[h]  (rows 0-63 = Q^T, 64-127 = K^T)
        # ZQ   = wzq[h].T @ xt[h]  (rows 0-63 = 0,   64-127 = Q^T)
        qkt = qkt_pool.tile([128, S], F32R, tag="qkt")
        zq = qkt_pool.tile([128, S], F32R, tag="zq")
        proj_state[h] = (qkt, zq)
        for dst, w_sb in ((qkt, wqk_sb), (zq, wzq_sb)):
            for sh in range(2):
                # single-bank psum tiles from the misc pool: keeps the next
                # head's projection off the scores pool, whose slots recycle
                # at exp speed
                p_ps = ps_misc.tile([128, 512], F32, tag="misc")
                nc.tensor.matmul(
                    p_ps,
                    w_sb[:, h, :],
                    xt_sb[h][:, 512 * sh : 512 * sh + 512],
                    start=True,
                    stop=True,
                )
                nc.vector.tensor_copy(dst[:, 512 * sh : 512 * sh + 512], p_ps)
                yield

        # V' per t-chunk: [128, 66]; 4 chunks per 1-bank psum tile.
        # vp tail-padded so lhsT slices [66c : 66c+128] stay in-bounds.
        vp = vp_pool.tile([128, NT * VW + 64], BF16, tag="vp")
        vp_sb[h] = vp
        nc.gpsimd.memset(vp[:, NT * VW : NT * VW + 64], 0.0)
        for half in range(2):
            vp_ps = ps_misc.tile([128, 4 * VW], F32, tag="misc")
            for i in range(4):
                tcn = 4 * half + i
                nc.tensor.matmul(
                    vp_ps[:, VW * i : VW * i + VW],
                    xt_sb[h][:, 128 * tcn : 128 * tcn + 128],
                    wv_sb[:, h, :],
                    start=True,
                    stop=True,
                )
            nc.vector.tensor_copy(
                vp[:, 4 * VW * half : 4 * VW * half + 4 * VW], vp_ps
            )
            yield

    def emit_sc_av(h, hp, nxt=None):
        # Interleave this head's scores (whose matmuls stall on exp freeing
        # PSUM slots — exp is ~2x slower than a matmul pair) with the
        # previous head's attn@V accumulation so PE stays busy.
        # attn@V: OT'[e', s] = sum_t V'[t, e'] attnT[t, s], via lhsT = vp
        # 128-wide slice (M=128: cols 0-64 real, 65.. garbage), rhs = attnT.
        ats = []
        ot_ps = vp = None
        if hp is not None:
            ot_ps = ps_ot.tile([128, S], F32, tag="ot")
            vp = vp_sb[hp]
        for tcn in range(NT):
            if hp is not None:
                for sh in range(2):
                    # M=98: near-smallest col count spanning all four PE
                    # column groups (fp32r requirement) -> shorter LDWEIGHTS
                    nc.tensor.matmul(
                        ot_ps[0:98, 512 * sh : 512 * sh + 512],
                        vp[:, VW * tcn : VW * tcn + 98],
                        at_tiles[hp][tcn][:, 512 * sh : 512 * sh + 512],
                        start=(tcn == 0),
                        stop=(tcn == NT - 1),
                    )
            if h is not None:
                qkt, zq = proj_state[h]
                sc_ps = ps_sc.tile([128, S], F32, tag="sc")
                lhsT = qkt[:, 128 * tcn : 128 * tcn + 128]  # [Q^T; K^T] chunk
                for sh in range(2):
                    nc.tensor.matmul(
                        sc_ps[:, 512 * sh : 512 * sh + 512],
                        lhsT,
                        zq[:, 512 * sh : 512 * sh + 512],
                        start=True,
                        stop=True,
                    )
                at = attn_pool.tile([128, S], BF16, tag="at")
                nc.scalar.activation(at, sc_ps, Exp)
                ats.append(at)
            if nxt is not None:
                next(nxt, None)
        if h is not None:
            at_tiles[h] = ats
        if hp is not None:
            ot = otsb_pool.tile([98, S], F32R, tag="ot_sb")
            nc.vector.tensor_copy(ot, ot_ps[0:98, :])
            ot_sb[hp] = ot
            del at_tiles[hp]
            del vp_sb[hp]

    def out_steps(h):
        # PE-transpose OT' back to [s, e] in 128-chunks; col 64 = 8*rowsum
        ot = ot_sb[h]
        for half in range(2):
            ott_ps = ps_misc.tile([128, 4 * 128], F32R, tag="misc")
            for i in range(4):
                scn = 4 * half + i
                nc.tensor.transpose(
                    ott_ps[:, 128 * i : 128 * i + 98],
                    ot[:, 128 * scn : 128 * scn + 128],
                    ident[0:98, 0:98],
                )
            ottv = ott_ps.bitcast(F32).rearrange("p (c w) -> p c w", w=128)
            rec = recip_pool.tile([128, 4], F32, tag="rec")
            nc.vector.reciprocal(rec, ottv[:, :, 64])
            rec_b = bass.AP(
                tensor=rec.tensor, offset=rec.offset, ap=list(rec.ap) + [[0, 64]]
            )
            nc.vector.tensor_mul(
                ysb[:, 4 * half : 4 * half + 4, 64 * h : 64 * h + 64],
                ottv[:, :, 0:64],
                rec_b,
            )
            yield
        del ot_sb[h]

    import itertools

    proj_state = {}
    for _ in proj_steps(0):
        pass
    for h in range(H + 2):
        cur = h if h < H else None
        prev = h - 1 if 1 <= h <= H else None
        gens = []
        if h + 1 < H:
            gens.append(proj_steps(h + 1))
        if 2 <= h and h - 2 < H:
            gens.append(out_steps(h - 2))
        nxt = itertools.chain(*gens) if gens else None
        if cur is not None or prev is not None:
            emit_sc_av(cur, prev, nxt)
        if nxt is not None:
            for _ in nxt:  # drain any remaining steps
                pass
        if cur is not None:
            proj_state.pop(h)
        if h in (6, 10, 12):
            # flush completed output columns while later heads compute,
            # shrinking the final DMA tail (out(h-2) done by iteration h)
            c0, c1 = {6: (0, 320), 10: (320, 576), 12: (576, 704)}[h]
            for scn in range(NT):
                eng = nc.sync if scn % 2 == 0 else nc.gpsimd
                eng.dma_start(
                    out=y_d[128 * scn : 128 * scn + 128, c0:c1],
                    in_=ysb[:, scn, c0:c1],
                )

    # ---- store ----------------------------------------------------------
    for scn in range(NT):
        eng = nc.sync if scn % 2 == 0 else nc.scalar
        eng.dma_start(
            out=y_d[128 * scn : 128 * scn + 128, 704:D],
            in_=ysb[:, scn, 704:D],
        )


# --------------------------------------------------------------------------
# host side
# --------------------------------------------------------------------------

_NC_CACHE = {}

LAST_EXEC_NS = None
LAST_RESULTS = None


def _get_nc():
    if "nc" not in _NC_CACHE:
        _NC_CACHE["nc"] = build_nc()
    return _NC_CACHE["nc"]


def prep_inputs(x, Wq, bq, Wk, bk, Wv, bv):
    """Host-side layout prep. Returns per-core input maps."""
    x = np.ascontiguousarray(np.asarray(x, dtype=np.float32))
    Wq, bq = np.asarray(Wq, np.float32), np.asarray(bq, np.float32)
    Wk, bk = np.asarray(Wk, np.float32), np.asarray(bk, np.float32)
    Wv, bv = np.asarray(Wv, np.float32), np.asarray(bv, np.float32)

    # xt: [B, H, 128, S]: rows 0-63 = x^T, row 64 = ones, rows 65-127 = 0
    # (zero-padded to K=128 so every matmul keeps the full PE array active —
    #  half-height matmuls trip the HAM activity monitor into throttling)
    xt = np.zeros((B, H, 128, S), np.float32)
    xt[:, :, :HD] = x.transpose(0, 2, 1).reshape(B, H, HD, S)
    xt[:, :, HD] = 1.0

    def stack2(Wa, ba, Wb, bb):
        w = np.zeros((H, 128, 128), np.float32)
        w[:, :HD, :HD] = Wa.transpose(0, 2, 1)
        w[:, :HD, HD:] = Wb.transpose(0, 2, 1)
        w[:, HD, :HD] = ba
        w[:, HD, HD:] = bb
        return w

    wqk = stack2(Wq, bq, Wk, bk)
    wzq = np.zeros((H, 128, 128), np.float32)
    wzq[:, :HD, HD:] = Wq.transpose(0, 2, 1)
    wzq[:, HD, HD:] = bq

    wv = np.zeros((H, 128, VW), np.float32)
    wv[:, :HD, :HD] = Wv.transpose(0, 2, 1)
    wv[:, HD, :HD] = bv
    wv[:, HD, HD] = 8.0  # ones col scaled by sqrt(HD) -> folds post-softmax /8

    ident = np.eye(128, dtype=np.float32)

    return [
        {"xt": xt[b], "wqk": wqk, "wzq": wzq, "wv": wv, "ident": ident}
        for b in range(B)
    ]


def kernel(x, Wq, bq, Wk, bk, Wv, bv):
    global LAST_EXEC_NS, LAST_RESULTS
    from concourse.bass_utils import run_bass_kernel_spmd

    nc = _get_nc()
    in_maps = prep_inputs(x, Wq, bq, Wk, bk, Wv, bv)
    trace = os.environ.get("KERNEL_TRACE", "0") == "1"
    res = run_bass_kernel_spmd(
        nc,
        in_maps,
        core_ids=list(range(B)),
        trace=trace,
    )
    LAST_EXEC_NS = res.exec_time_ns
    LAST_RESULTS = res
    y = np.stack([res.results[b]["y"] for b in range(B)], axis=0)
    return y.astype(np.float32)
